# revision 1
# baseline (speedup 1.0000x reference)
"""Trainium2 Bass kernel for BertAttention (B=16, S=1024, H=768, 12 heads).

Strategy: data-parallel over batch across 8 NeuronCores (2 batch rows per
core), no collectives.  Per core:
  - cast x / weights to bf16 in DRAM (SWDGE cast DMA), hardware DMA-transpose
    into SBUF to get contraction-dim-on-partition layouts (zero PE cost).
  - QKV projections as lhsT.T @ rhs matmuls (bf16, fp32 PSUM accum), Q/K in
    transposed [feature, token] layout, V in natural [token, feature] layout.
  - attention per head-pair: head_dim=64 so two heads run concurrently in the
    128x128 PE array via row/col tile_position packing.
  - softmax: no max-subtraction needed (scores are O(1)); exp on ScalarE with
    the 1/sqrt(64) scale folded in; additive mask applied exactly as a
    multiplicative exp(mask) folded into V rows and the denominator lhsT.
  - denominators via ones-matmul (sum over k lands broadcast across
    partitions); reciprocal + multiply folded into the PSUM->SBUF copy.
  - output projection + residual + LayerNorm (bn_stats/bn_aggr, one batched
    Sqrt per batch row to avoid ACT table thrashing with exp).

Workaround: this container's walrus accepts only ONE sync wait per
instruction; a post-pass splits multi-wait instructions into single-wait
NOPs.
"""

import numpy as np

import concourse.bass as bass
import concourse.mybir as mybir
import concourse.tile as tile
from concourse.tile import add_dep_helper
from concourse.masks import make_identity

P = 128
H = 768
NH = 12
HD = 64
S = 1024
B = 16
NCORES = 8
BPC = B // NCORES  # batch rows per core = 2
IO_T = H // P      # 6 contraction tiles
KO_T = S // P      # 8 k tiles per sequence
SCALE = 1.0 / 8.0  # 1/sqrt(64)
LN_EPS = 1e-12

F32 = mybir.dt.float32
BF16 = mybir.dt.bfloat16
AF = mybir.ActivationFunctionType
OP = mybir.AluOpType


def _split_multi_waits(nc):
    """walrus here rejects >1 sync wait per instruction; hoist extras into
    single-wait NOPs on the same engine immediately before."""
    n = 0
    for blk in nc.m.functions[0].blocks:
        insts = blk.instructions
        new = []
        changed = False
        for inst in insts:
            si = inst.sync_info
            waits = list(si.on_wait) if si and si.on_wait else []
            if len(waits) > 1:
                changed = True
                for k, w in enumerate(waits[:-1]):
                    n += 1
                    new.append(
                        mybir.InstNoOp(
                            name=f"ws-{blk.name}-{inst.name}-{k}",
                            engine=inst.engine,
                            sync_info=mybir.SyncInfo(on_wait=[w], on_update=[]),
                        )
                    )
                inst.sync_info = mybir.SyncInfo(
                    on_wait=[waits[-1]], on_update=list(si.on_update)
                )
            new.append(inst)
        if changed:
            blk.instructions = new
    return n


def _bcast_ap(ap, parts=P):
    """Partition-broadcast view of a 1-D DRAM AP: [parts, len]."""
    return bass.AP(tensor=ap.tensor, offset=ap.offset, ap=[[0, parts]] + list(ap.ap))


def build_bass():
    nc = bass.Bass()

    hs = nc.declare_dram_parameter("hs", [BPC, S, H], F32, isOutput=False)
    msk = nc.declare_dram_parameter("msk", [BPC, S], F32, isOutput=False)
    qw = nc.declare_dram_parameter("qw", [H, H], F32, isOutput=False)
    kw = nc.declare_dram_parameter("kw", [H, H], F32, isOutput=False)
    vw = nc.declare_dram_parameter("vw", [H, H], F32, isOutput=False)
    ow = nc.declare_dram_parameter("ow", [H, H], F32, isOutput=False)
    qb = nc.declare_dram_parameter("qb", [H], F32, isOutput=False)
    kb = nc.declare_dram_parameter("kb", [H], F32, isOutput=False)
    vb = nc.declare_dram_parameter("vb", [H], F32, isOutput=False)
    ob = nc.declare_dram_parameter("ob", [H], F32, isOutput=False)
    gamma = nc.declare_dram_parameter("gamma", [H], F32, isOutput=False)
    beta = nc.declare_dram_parameter("beta", [H], F32, isOutput=False)
    out = nc.declare_dram_parameter("out", [BPC, S, H], F32, isOutput=True)

    from contextlib import ExitStack

    with tile.TileContext(nc) as tc:
        with ExitStack() as ctx:
            _build_tile(
                ctx, tc, nc, hs, msk, qw, kw, vw, ow, qb, kb, vb, ob, gamma, beta, out
            )

    _split_multi_waits(nc)
    return nc


def _build_tile(ctx, tc, nc, hs, msk, qw, kw, vw, ow, qb, kb, vb, ob, gamma, beta, out):
    dram = ctx.enter_context(tc.tile_pool(name="dram", bufs=1, space="DRAM"))
    consts = ctx.enter_context(tc.tile_pool(name="consts", bufs=1))
    perb = ctx.enter_context(tc.tile_pool(name="perb", bufs=1))
    xt_pool = ctx.enter_context(tc.tile_pool(name="xt", bufs=2))
    qk_pool = ctx.enter_context(tc.tile_pool(name="qk", bufs=6))
    v_pool = ctx.enter_context(tc.tile_pool(name="v", bufs=2))
    pt_pool = ctx.enter_context(tc.tile_pool(name="pt", bufs=3))
    r_pool = ctx.enter_context(tc.tile_pool(name="rcp", bufs=2))
    xres_pool = ctx.enter_context(tc.tile_pool(name="xres", bufs=2))
    s_pool = ctx.enter_context(tc.tile_pool(name="s", bufs=4))
    o_pool = ctx.enter_context(tc.tile_pool(name="o", bufs=2))
    ln_pool = ctx.enter_context(tc.tile_pool(name="ln", bufs=4))

    ps_proj = ctx.enter_context(tc.tile_pool(name="psp", bufs=2, space="PSUM"))
    ps_acc = ctx.enter_context(tc.tile_pool(name="psa", bufs=1, space="PSUM"))
    ps_big = ctx.enter_context(tc.tile_pool(name="psb", bufs=2, space="PSUM"))

    # ---- constants / weight preparation -------------------------------
    # bf16 copies of weights/x in DRAM (SWDGE cast), per-column-block so the
    # HWDGE transposes pipeline behind the casts instead of waiting for full
    # tensors.  Ordering puts qw and x[b=0] first: the first projection
    # matmuls only wait ~a couple of blocks.
    w_bf = {}
    wT = {}
    for name in ("q", "k", "v", "o"):
        w_bf[name] = dram.tile([H, H], BF16, tag=f"wbf_{name}", name=f"wbf_{name}")
        wT[name] = consts.tile([P, IO_T, H], BF16, tag=f"wT_{name}", name=f"wT_{name}")
    x_bf = dram.tile([BPC, S, H], BF16)
    wsrc = {"q": qw, "k": kw, "v": vw, "o": ow}

    def transpose_w(name):
        for io in range(IO_T):
            nc.sync.dma_start_transpose(
                wT[name][:, io, :], w_bf[name][:, io * P : (io + 1) * P]
            )

    # contiguous whole-tensor casts flattened to 1-D (sprays across all 16
    # DMA queues instead of per-row descriptors), most-urgent first; the
    # per-io HWDGE transposes pipeline behind them
    def cast_flat(dst_ap, src_ap):
        nc.gpsimd.dma_start(out=dst_ap, in_=src_ap)

    xTs = []
    for b in range(BPC):
        xTs.append(xt_pool.tile([P, IO_T, S], BF16, tag="xT", name=f"xT_{b}"))

    def transpose_x(b):
        for io in range(IO_T):
            nc.sync.dma_start_transpose(
                xTs[b][:, io, :], x_bf[b, :, io * P : (io + 1) * P]
            )

    cast_flat(w_bf["q"][:, :], qw[:, :])
    cast_flat(x_bf[0], hs[:, :, :][0])
    transpose_w("q")
    transpose_x(0)
    cast_flat(w_bf["k"][:, :], kw[:, :])
    transpose_w("k")
    cast_flat(w_bf["v"][:, :], vw[:, :])
    transpose_w("v")
    cast_flat(x_bf[1], hs[:, :, :][1])
    transpose_x(1)
    cast_flat(w_bf["o"][:, :], ow[:, :])
    transpose_w("o")

    gamma_bc = consts.tile([P, H], F32, tag="gamma_bc")
    nc.gpsimd.dma_start(out=gamma_bc, in_=_bcast_ap(gamma[:]))
    beta_bc = consts.tile([P, H], F32, tag="beta_bc")
    nc.gpsimd.dma_start(out=beta_bc, in_=_bcast_ap(beta[:]))

    qb_sb = consts.tile([P, IO_T], F32, tag="qb")
    nc.sync.dma_start(out=qb_sb, in_=qb[:].rearrange("(o p) -> p o", p=P))
    kb_sb = consts.tile([P, IO_T], F32, tag="kb")
    nc.sync.dma_start(out=kb_sb, in_=kb[:].rearrange("(o p) -> p o", p=P))

    vb_row = consts.tile([1, H], BF16, tag="vb_row")
    nc.gpsimd.dma_start(out=vb_row, in_=vb[:][None, :])
    ob_row = consts.tile([1, H], BF16, tag="ob_row")
    nc.gpsimd.dma_start(out=ob_row, in_=ob[:][None, :])

    eps_sb = consts.tile([P, 1], F32, tag="eps")
    nc.vector.memset(eps_sb, LN_EPS)
    ones64 = consts.tile([P, HD], F32, tag="ones64")
    nc.vector.memset(ones64, 1.0)
    ones_row = consts.tile([1, P], BF16, tag="ones_row")
    nc.vector.memset(ones_row, 1.0)

    HP = NH // 2  # 6 head pairs
    QT_CH = 512   # q chunk (free dim of attention matmuls)
    NQ = S // QT_CH  # 2

    for b in range(BPC):
        # ---- per-b prep ------------------------------------------------
        xT = xTs[b]

        mask_sb = perb.tile([P, KO_T], F32, tag="mask")
        nc.sync.dma_start(out=mask_sb, in_=msk[:, :][b].rearrange("(o p) -> p o", p=P))
        em_sb = perb.tile([P, KO_T], F32, tag="em")
        nc.scalar.activation(out=em_sb, in_=mask_sb, func=AF.Exp)
        em_lhsT = perb.tile([P, KO_T, HD], BF16, tag="em_lhsT")
        for ko in range(KO_T):
            nc.vector.tensor_scalar_mul(
                out=em_lhsT[:, ko, :], in0=ones64, scalar1=em_sb[:, ko : ko + 1]
            )

        # ---- QKV projections ------------------------------------------
        QTs = {}
        KTs = {}
        for store, wname, bias in ((QTs, "q", qb_sb), (KTs, "k", kb_sb)):
            for jo in range(IO_T):
                store[jo] = qk_pool.tile(
                    [P, S], BF16, tag="QT" if wname == "q" else "KT",
                    name=f"{wname}T_{jo}",
                )
            for tt in range(S // 512):
                for jo in range(IO_T):
                    ps = ps_proj.tile([P, 512], F32, tag="proj")
                    for io in range(IO_T):
                        nc.tensor.matmul(
                            ps,
                            lhsT=wT[wname][:, io, jo * P : (jo + 1) * P],
                            rhs=xT[:, io, tt * 512 : (tt + 1) * 512],
                            start=(io == 0),
                            stop=(io == IO_T - 1),
                        )
                    nc.vector.tensor_scalar_add(
                        out=store[jo][:, tt * 512 : (tt + 1) * 512],
                        in0=ps,
                        scalar1=bias[:, jo : jo + 1],
                    )

        V = v_pool.tile([P, KO_T, H], BF16, tag="V")
        for t8 in range(KO_T):
            for jh in range(2):
                ps = ps_proj.tile([P, 512], F32, tag="proj")
                for io in range(IO_T):
                    nc.tensor.matmul(
                        ps[:, 0:384],
                        lhsT=xT[:, io, t8 * P : (t8 + 1) * P],
                        rhs=wT["v"][:, io, jh * 384 : (jh + 1) * 384],
                        start=(io == 0),
                        stop=False,
                    )
                nc.tensor.matmul(
                    ps[:, 0:384],
                    lhsT=ones_row,
                    rhs=vb_row[:, jh * 384 : (jh + 1) * 384],
                    start=False,
                    stop=True,
                )
                # copy + exp(mask) row scaling (exact multiplicative mask)
                nc.vector.tensor_scalar_mul(
                    out=V[:, t8, jh * 384 : (jh + 1) * 384],
                    in0=ps[:, 0:384],
                    scalar1=em_sb[:, t8 : t8 + 1],
                )

        # ---- attention per head pair ----------------------------------
        ctxT = perb.tile([P, HP, S], BF16, tag="ctxT")
        for hp in range(HP):
            for qt in range(NQ):
                qsl = slice(qt * QT_CH, (qt + 1) * QT_CH)
                ptA = pt_pool.tile([P, KO_T, QT_CH], BF16, tag="ptA")
                ptB = pt_pool.tile([P, KO_T, QT_CH], BF16, tag="ptB")
                # scores^T -> exp, in chunks of 2 k-tiles (2 PSUM banks)
                for kc in range(KO_T // 2):
                    for pt_dst, lo in ((ptA, 0), (ptB, HD)):
                        sc = ps_big.tile([P, 2, 512], F32, tag="sc")
                        for k2 in range(2):
                            ko = kc * 2 + k2
                            nc.tensor.matmul(
                                sc[:, k2, :],
                                lhsT=KTs[hp][lo : lo + HD, ko * P : (ko + 1) * P],
                                rhs=QTs[hp][lo : lo + HD, qsl],
                                start=True,
                                stop=True,
                            )
                        nc.scalar.activation(
                            out=pt_dst[:, kc * 2 : kc * 2 + 2, :],
                            in_=sc,
                            func=AF.Exp,
                            scale=SCALE,
                        )
                # denominators + P@V, two heads col-packed per bank
                sums = ps_acc.tile([P, QT_CH], F32, tag="sums")
                ctxp = ps_acc.tile([P, QT_CH], F32, tag="ctxp")
                first_s = {}
                for ko in range(KO_T):
                    mmA = nc.tensor.matmul(
                        sums[0:HD, :],
                        lhsT=em_lhsT[:, ko, :],
                        rhs=ptA[:, ko, :],
                        start=(ko == 0),
                        stop=(ko == KO_T - 1),
                        tile_position=(0, 0),
                    )
                    mmB = nc.tensor.matmul(
                        sums[HD:P, :],
                        lhsT=em_lhsT[:, ko, :],
                        rhs=ptB[:, ko, :],
                        start=False,
                        stop=(ko == KO_T - 1),
                        tile_position=(0, HD),
                        skip_group_check=True,
                    )
                    if ko == 0:
                        add_dep_helper(mmB.ins, mmA.ins, sync=False, reason="bank clear order")
                    mmC = nc.tensor.matmul(
                        ctxp[0:HD, :],
                        lhsT=V[:, ko, hp * P : hp * P + HD],
                        rhs=ptA[:, ko, :],
                        start=(ko == 0),
                        stop=(ko == KO_T - 1),
                        tile_position=(0, 0),
                    )
                    mmD = nc.tensor.matmul(
                        ctxp[HD:P, :],
                        lhsT=V[:, ko, hp * P + HD : (hp + 1) * P],
                        rhs=ptB[:, ko, :],
                        start=False,
                        stop=(ko == KO_T - 1),
                        tile_position=(0, HD),
                        skip_group_check=True,
                    )
                    if ko == 0:
                        add_dep_helper(mmD.ins, mmC.ins, sync=False, reason="bank clear order")
                rcp = r_pool.tile([P, QT_CH], F32, tag="rcp")
                nc.vector.reciprocal(out=rcp, in_=sums)
                nc.vector.tensor_tensor(
                    out=ctxT[:, hp, qsl], in0=ctxp, in1=rcp, op=OP.mult
                )

        # ---- output projection + residual + layernorm -----------------
        mv_all = ln_pool.tile([P, KO_T, 2], F32, tag="mv")
        rstd = ln_pool.tile([P, KO_T], F32, tag="rstd")
        s_tiles = []
        for t8 in range(KO_T):
            xres = xres_pool.tile([P, H], F32, tag="xres")
            nc.gpsimd.dma_start(out=xres, in_=hs[b, t8 * P : (t8 + 1) * P, :])
            s_t = s_pool.tile([P, H], F32, tag="s")
            for jh in range(2):
                ps = ps_proj.tile([P, 512], F32, tag="proj")
                for io in range(IO_T):
                    nc.tensor.matmul(
                        ps[:, 0:384],
                        lhsT=ctxT[:, io, t8 * P : (t8 + 1) * P],
                        rhs=wT["o"][:, io, jh * 384 : (jh + 1) * 384],
                        start=(io == 0),
                        stop=False,
                    )
                nc.tensor.matmul(
                    ps[:, 0:384],
                    lhsT=ones_row,
                    rhs=ob_row[:, jh * 384 : (jh + 1) * 384],
                    start=False,
                    stop=True,
                )
                nc.vector.tensor_tensor(
                    out=s_t[:, jh * 384 : (jh + 1) * 384],
                    in0=ps[:, 0:384],
                    in1=xres[:, jh * 384 : (jh + 1) * 384],
                    op=OP.add,
                )
            stats = ln_pool.tile([P, 3, 6], F32, tag="stats")
            for sg in range(3):
                nc.vector.bn_stats(
                    out=stats[:, sg, :], in_=s_t[:, sg * 256 : (sg + 1) * 256]
                )
            nc.vector.bn_aggr(out=mv_all[:, t8, :], in_=stats)
            s_tiles.append(s_t)

            # every 4 tiles: batched rsqrt then normalize + store (keeps the
            # s-tile pool small and lets the tail overlap PE work)
            if t8 % 4 == 3:
                h0 = t8 - 3
                nc.scalar.activation(
                    out=rstd[:, h0 : t8 + 1],
                    in_=mv_all[:, h0 : t8 + 1, 1],
                    func=AF.Sqrt,
                    bias=eps_sb,
                    scale=1.0,
                )
                nc.vector.reciprocal(
                    out=rstd[:, h0 : t8 + 1], in_=rstd[:, h0 : t8 + 1]
                )
                for u8 in range(h0, t8 + 1):
                    o_t = o_pool.tile([P, H], F32, tag="o")
                    nc.vector.tensor_scalar(
                        out=o_t,
                        in0=s_tiles[u8],
                        scalar1=mv_all[:, u8, 0:1],
                        scalar2=rstd[:, u8 : u8 + 1],
                        op0=OP.subtract,
                        op1=OP.mult,
                    )
                    nc.vector.tensor_tensor(out=o_t, in0=o_t, in1=gamma_bc, op=OP.mult)
                    nc.vector.tensor_tensor(out=o_t, in0=o_t, in1=beta_bc, op=OP.add)
                    nc.sync.dma_start(out=out[b, u8 * P : (u8 + 1) * P, :], in_=o_t)


_nc_cache = None


def _get_nc():
    global _nc_cache
    if _nc_cache is None:
        _nc_cache = build_bass()
    return _nc_cache


def kernel(**inputs):
    from concourse.bass_utils import run_bass_kernel_spmd

    hs = np.asarray(inputs["hidden_states"], np.float32)
    mask = np.asarray(inputs["attention_mask"], np.float32).reshape(B, S)
    names = {
        "qw": inputs["qw"], "kw": inputs["kw"], "vw": inputs["vw"], "ow": inputs["ow"],
        "qb": inputs["qb"], "kb": inputs["kb"], "vb": inputs["vb"], "ob": inputs["ob"],
        "gamma": inputs["gamma"], "beta": inputs["beta"],
    }
    shared = {k: np.ascontiguousarray(np.asarray(v, np.float32)) for k, v in names.items()}
    in_maps = []
    for c in range(NCORES):
        m = dict(shared)
        m["hs"] = np.ascontiguousarray(hs[c * BPC : (c + 1) * BPC])
        m["msk"] = np.ascontiguousarray(mask[c * BPC : (c + 1) * BPC])
        in_maps.append(m)

    # A rare per-process DMA race can corrupt a core's staging buffer, which
    # surfaces as NaN/Inf (garbage scores overflow exp).  It is sticky per
    # module load, so after two failed re-runs rebuild the Bass module (new
    # NEFF load + fresh allocations) and try again.
    global _nc_cache
    out = None
    for attempt in range(6):
        res = run_bass_kernel_spmd(_get_nc(), in_maps, core_ids=list(range(NCORES)))
        out = np.concatenate([res.results[c]["out"] for c in range(NCORES)], axis=0)
        if np.isfinite(out).all():
            break
        if attempt >= 1:
            _nc_cache = None  # force a rebuild / fresh module
    return out



# revision 4
# speedup vs baseline: 1.9095x; 1.9095x over previous
"""Trainium2 Bass kernel for BertAttention (B=16, S=1024, H=768, 12 heads).

Data-parallel over batch across 8 NeuronCores (2 rows/core), no collectives.

v2 strategy (fp8 DoubleRow everywhere):
  - Host precomputes weight layouts: transposed, x32-scaled, fp8(e4m3)-cast,
    and (for q/k) column-permuted so the projection PSUM partitions land
    directly in the DoubleRow [32p x 2ksub] head layout.  Host also uploads
    x^T in fp8 (x feeds matmuls only via x^T; the f32 x is uploaded
    separately for the residual).
  - All projections / scores / PV run as fp8e4 DoubleRow matmuls
    (2 contraction rows per partition, 0.5 PE cycles per output row).
  - Softmax denominators are FREE: V tiles carry a 65th column holding
    em/8 (em = exp(mask)), so the PV matmul's PSUM row 64 is sums/8.
    lhsT width is padded to 96 (dual-fp8 ldweights requires multiples of 32).
  - Normalization 1/sums is applied to ctx^T via a DRAM-roundtrip partition
    broadcast of the bf16 reciprocal (DMA stride-0), folded into the
    PSUM->SBUF fp8 cast of ctx^T (x8 scale folded into the reciprocal).
  - O-projection contracts ctx^T [64p x 2ksub(head)] DoubleRow; the 1/8192
    descale + residual add is one fused scalar_tensor_tensor on DVE.
  - LayerNorm via bn_stats/bn_aggr with batched Sqrt (as v1).
  - K/V projection epilogues run on Pool (gpsimd) to keep DVE under the
    Activation (exp) roofline; exp on ScalarE is the bottleneck engine.

Specialized variant assumes qb=kb=vb=ob=0, mask=0, gamma=1, beta=0 (verified
host-side; a general variant is built lazily if the check fails).

Workaround: this container's walrus accepts only ONE sync wait per
instruction; a post-pass splits multi-wait instructions into single-wait
NOPs.
"""

import numpy as np

import concourse.bass as bass
import concourse.mybir as mybir
import concourse.tile as tile

P = 128
H = 768
NH = 12
HD = 64
S = 1024
B = 16
NCORES = 8
BPC = B // NCORES  # 2
IO_T = H // P      # 6
KO_T = S // P      # 8
WS = 32.0          # weight scale folded into fp8 weights
CS = 8.0           # ctx scale: em column = 1/CS so rcp = CS/sums
OS = 1.0 / 8192.0  # o-proj descale: 1/(CS*WS*WS)
EXP_SCALE = 1.0 / 8192.0  # scores descale: 1/(8*WS*WS)
LN_EPS = 1e-12

F32 = mybir.dt.float32
BF16 = mybir.dt.bfloat16
FP8 = mybir.dt.float8e4
AF = mybir.ActivationFunctionType
OP = mybir.AluOpType
DR = mybir.MatmulPerfMode.DoubleRow


def _split_multi_waits(nc):
    """walrus here rejects >1 sync wait per instruction; hoist extras into
    single-wait NOPs on the same engine immediately before."""
    n = 0
    for blk in nc.m.functions[0].blocks:
        insts = blk.instructions
        new = []
        changed = False
        for inst in insts:
            si = inst.sync_info
            waits = list(si.on_wait) if si and si.on_wait else []
            if len(waits) > 1:
                changed = True
                for k, w in enumerate(waits[:-1]):
                    n += 1
                    new.append(
                        mybir.InstNoOp(
                            name=f"ws-{blk.name}-{inst.name}-{k}",
                            engine=inst.engine,
                            sync_info=mybir.SyncInfo(on_wait=[w], on_update=[]),
                        )
                    )
                inst.sync_info = mybir.SyncInfo(
                    on_wait=[waits[-1]], on_update=list(si.on_update)
                )
            new.append(inst)
        if changed:
            blk.instructions = new
    return n


def _bcast_ap(ap, parts=P):
    """Partition-broadcast view of a DRAM AP row: [parts, len]."""
    return bass.AP(tensor=ap.tensor, offset=ap.offset, ap=[[0, parts]] + list(ap.ap)[1:])


def build_bass(general=False):
    nc = bass.Bass()

    hs = nc.declare_dram_parameter("hs", [BPC, S, H], F32, isOutput=False)
    xt8 = nc.declare_dram_parameter("xt8", [BPC, H, S], FP8, isOutput=False)
    wq8 = nc.declare_dram_parameter("wq8", [P, IO_T, H], FP8, isOutput=False)
    wk8 = nc.declare_dram_parameter("wk8", [P, IO_T, H], FP8, isOutput=False)
    wv8 = nc.declare_dram_parameter("wv8", [P, IO_T, H], FP8, isOutput=False)
    wo8 = nc.declare_dram_parameter("wo8", [HD, NH, H], FP8, isOutput=False)
    out = nc.declare_dram_parameter("out", [BPC, S, H], F32, isOutput=True)
    g = {}
    if general:
        g["qb"] = nc.declare_dram_parameter("qb32p", [P, IO_T], F32, isOutput=False)
        g["kb"] = nc.declare_dram_parameter("kb32p", [P, IO_T], F32, isOutput=False)
        g["vb"] = nc.declare_dram_parameter("vb32", [1, H], F32, isOutput=False)
        g["ob"] = nc.declare_dram_parameter("ob8192", [1, H], F32, isOutput=False)
        g["msk"] = nc.declare_dram_parameter("msk", [BPC, S], F32, isOutput=False)
        g["gamma"] = nc.declare_dram_parameter("gamma", [H], F32, isOutput=False)
        g["beta"] = nc.declare_dram_parameter("beta", [H], F32, isOutput=False)

    from contextlib import ExitStack

    with tile.TileContext(nc) as tc:
        with ExitStack() as ctx:
            _build_tile(ctx, tc, nc, hs, xt8, wq8, wk8, wv8, wo8, out, g)

    _split_multi_waits(nc)
    return nc


def _build_tile(ctx, tc, nc, hs, xt8, wq8, wk8, wv8, wo8, out, g):
    general = bool(g)

    consts = ctx.enter_context(tc.tile_pool(name="consts", bufs=1))
    dram = ctx.enter_context(tc.tile_pool(name="dram", bufs=4, space="DRAM"))
    xres_pool = ctx.enter_context(tc.tile_pool(name="xres", bufs=2))
    pt_pool = ctx.enter_context(tc.tile_pool(name="pt", bufs=3))
    rcp_pool = ctx.enter_context(tc.tile_pool(name="rcp", bufs=3))
    bc_pool = ctx.enter_context(tc.tile_pool(name="bc", bufs=3))
    s_pool = ctx.enter_context(tc.tile_pool(name="s", bufs=5))
    o_pool = ctx.enter_context(tc.tile_pool(name="o", bufs=2))
    ln_pool = ctx.enter_context(tc.tile_pool(name="ln", bufs=1))

    ps_proj = ctx.enter_context(tc.tile_pool(name="psp", bufs=2, space="PSUM"))
    ps_sc = ctx.enter_context(tc.tile_pool(name="pssc", bufs=2, space="PSUM"))
    ps_pv = ctx.enter_context(tc.tile_pool(name="pspv", bufs=2, space="PSUM"))

    # ---- weights / constants into SBUF --------------------------------
    wq = consts.tile([P, IO_T, H], FP8, tag="wq")
    nc.sync.dma_start(out=wq, in_=wq8[:, :, :])
    wk = consts.tile([P, IO_T, H], FP8, tag="wk")
    nc.sync.dma_start(out=wk, in_=wk8[:, :, :])
    wv = consts.tile([P, IO_T, H], FP8, tag="wv")
    nc.sync.dma_start(out=wv, in_=wv8[:, :, :])
    wo = consts.tile([HD, NH, H], FP8, tag="wo")
    nc.sync.dma_start(out=wo, in_=wo8[:, :, :])

    eps_sb = consts.tile([P, 1], F32, tag="eps")
    nc.vector.memset(eps_sb, LN_EPS)
    ones1 = consts.tile([1, HD], BF16, tag="ones1")
    nc.vector.memset(ones1, 1.0)

    if general:
        ones_row = consts.tile([1, P], BF16, tag="ones_row")
        nc.vector.memset(ones_row, 1.0)
        qb_sb = consts.tile([P, IO_T], F32, tag="qb")
        nc.sync.dma_start(out=qb_sb, in_=g["qb"][:, :])
        kb_sb = consts.tile([P, IO_T], F32, tag="kb")
        nc.sync.dma_start(out=kb_sb, in_=g["kb"][:, :])
        vb_row = consts.tile([1, H], BF16, tag="vb_row")
        nc.gpsimd.dma_start(out=vb_row, in_=g["vb"][:, :])
        ob_row = consts.tile([1, H], BF16, tag="ob_row")
        nc.gpsimd.dma_start(out=ob_row, in_=g["ob"][:, :])
        gamma_bc = consts.tile([P, H], F32, tag="gamma_bc")
        nc.gpsimd.dma_start(out=gamma_bc, in_=_bcast_ap(g["gamma"][None, :]))
        beta_bc = consts.tile([P, H], F32, tag="beta_bc")
        nc.gpsimd.dma_start(out=beta_bc, in_=_bcast_ap(g["beta"][None, :]))
        ones12 = consts.tile([P, NH], F32, tag="ones12")
        nc.vector.memset(ones12, 1.0)

    # ---- per-b persistent tiles ---------------------------------------
    x8Ts, Q8s, K8s, V8s, ctx8s, em8s = [], [], [], [], [], []
    for b in range(BPC):
        x8Ts.append(consts.tile([P, IO_T, S], FP8, tag=f"x8_{b}", name=f"x8_{b}"))
        Q8s.append(consts.tile([P, 3, 2, S], FP8, tag=f"q8_{b}", name=f"q8_{b}"))
        K8s.append(consts.tile([P, 3, 2, S], FP8, tag=f"k8_{b}", name=f"k8_{b}"))
        V8s.append(consts.tile([P, KO_T, NH * 96], FP8, tag=f"v8_{b}", name=f"v8_{b}"))
        ctx8s.append(consts.tile([HD, NH, S], FP8, tag=f"c8_{b}", name=f"c8_{b}"))
        em8s.append(None)

    def load_x8(b):
        for io in range(IO_T):
            nc.sync.dma_start(
                out=x8Ts[b][:, io, :], in_=xt8[b, io * P : (io + 1) * P, :]
            )
        if general:
            em8 = consts.tile([P, KO_T], F32, tag=f"em8_{b}")
            msk_sb = consts.tile([P, KO_T], F32, tag=f"msk_{b}")
            nc.sync.dma_start(
                out=msk_sb, in_=g["msk"][:, :][b].rearrange("(o p) -> p o", p=P)
            )
            # em/8 = exp(mask - ln 8)
            nc.scalar.activation(out=em8, in_=msk_sb, func=AF.Exp, bias=-2.0794415416798357)
            em8s[b] = em8

    def proj_qk(b):
        x8 = x8Ts[b]
        for wtile, store, bias in ((wq, Q8s[b], "qb"), (wk, K8s[b], "kb")):
            on_pool = wtile is wk
            for tt in range(2):
                for jo in range(IO_T):
                    ps = ps_proj.tile([P, 512], F32, tag="proj")
                    for i2 in range(3):
                        nc.tensor.matmul(
                            ps,
                            lhsT=wtile[:, 2 * i2 : 2 * i2 + 2, jo * P : (jo + 1) * P],
                            rhs=x8[:, 2 * i2 : 2 * i2 + 2, tt * 512 : (tt + 1) * 512],
                            start=(i2 == 0),
                            stop=(i2 == 2),
                            perf_mode=DR,
                        )
                    dst = store[:, jo // 2, jo % 2, tt * 512 : (tt + 1) * 512]
                    eng = nc.gpsimd if on_pool else nc.vector
                    if general:
                        bsb = qb_sb if bias == "qb" else kb_sb
                        eng.tensor_scalar_add(out=dst, in0=ps, scalar1=bsb[:, jo : jo + 1])
                    else:
                        eng.tensor_copy(out=dst, in_=ps)

    def proj_v(b):
        x8 = x8Ts[b]
        V8 = V8s[b]
        # em column (row k scale em/CS) + implicit junk pad cols 65..95
        if general:
            for t8 in range(KO_T):
                nc.vector.tensor_scalar_mul(
                    out=V8[:, t8, :].rearrange("p (h c) -> p h c", h=NH)[:, :, HD : HD + 1],
                    in0=ones12,
                    scalar1=em8s[b][:, t8 : t8 + 1],
                )
        else:
            nc.vector.memset(
                V8[:, :, :].rearrange("p k (h c) -> p k h c", h=NH)[:, :, :, HD : HD + 1],
                1.0 / CS,
            )
        for t8 in range(KO_T):
            for jh in range(2):
                ps = ps_proj.tile([P, 512], F32, tag="proj")
                for i2 in range(3):
                    nc.tensor.matmul(
                        ps[:, 0:384],
                        lhsT=x8[:, 2 * i2 : 2 * i2 + 2, t8 * P : (t8 + 1) * P],
                        rhs=wv[:, 2 * i2 : 2 * i2 + 2, jh * 384 : (jh + 1) * 384],
                        start=(i2 == 0),
                        stop=(i2 == 2) and not general,
                        perf_mode=DR,
                    )
                if general:
                    nc.tensor.matmul(
                        ps[:, 0:384],
                        lhsT=ones_row,
                        rhs=vb_row[:, jh * 384 : (jh + 1) * 384],
                        start=False,
                        stop=True,
                    )
                dst = (
                    V8[:, t8, 576 * jh : 576 * jh + 576]
                    .rearrange("p (u e) -> p u e", u=6)[:, :, 0:HD]
                )
                if general:
                    nc.gpsimd.tensor_scalar_mul(
                        out=dst, in0=ps[:, 0:384], scalar1=em8s[b][:, t8 : t8 + 1]
                    )
                else:
                    nc.gpsimd.tensor_copy(out=dst, in_=ps[:, 0:384])

    def attn(b, qt):
        qsl = slice(qt * 512, (qt + 1) * 512)
        for j in range(3):
            for i in range(4):
                h = 4 * j + i
                pt8 = pt_pool.tile([P, KO_T, 512], FP8, tag="pt")
                for kc in range(4):
                    sc = ps_sc.tile([P, 2, 512], F32, tag="sc")
                    for k2 in range(2):
                        ko = 2 * kc + k2
                        nc.tensor.matmul(
                            sc[:, k2, :],
                            lhsT=K8s[b][32 * i : 32 * i + 32, j, :, ko * P : (ko + 1) * P],
                            rhs=Q8s[b][32 * i : 32 * i + 32, j, :, qsl],
                            start=True,
                            stop=True,
                            perf_mode=DR,
                            tile_position=(32 * i, 0),
                        )
                    nc.scalar.activation(
                        out=pt8[:, 2 * kc : 2 * kc + 2, :],
                        in_=sc,
                        func=AF.Exp,
                        scale=EXP_SCALE,
                    )
                pv = ps_pv.tile([96, 512], F32, tag="pv")
                for d2 in range(4):
                    nc.tensor.matmul(
                        pv,
                        lhsT=V8s[b][:, 2 * d2 : 2 * d2 + 2, 96 * h : 96 * h + 96],
                        rhs=pt8[:, 2 * d2 : 2 * d2 + 2, :],
                        start=(d2 == 0),
                        stop=(d2 == 3),
                        perf_mode=DR,
                    )
                # row 64 = sums/CS ; rcp = CS/sums via bf16 reciprocal
                rcp = rcp_pool.tile([1, 512], BF16, tag="rcp")
                with nc.allow_low_precision(reason="bf16 softmax denom recip"):
                    nc.vector.reciprocal(out=rcp, in_=pv[HD : HD + 1, :])
                rcpd = dram.tile([1, 512], BF16, tag="rcpd")
                nc.sync.dma_start(out=rcpd, in_=rcp)
                bc = bc_pool.tile([HD, 512], BF16, tag="bc")
                nc.sync.dma_start(out=bc, in_=_bcast_ap(rcpd[:, :], parts=HD))
                nc.vector.tensor_tensor(
                    out=ctx8s[b][:, h, qsl], in0=pv[0:HD, :], in1=bc, op=OP.mult
                )

    def oproj_ln(b, t8s, lnst):
        ctx8 = ctx8s[b]
        for t8 in t8s:
            xres = xres_pool.tile([P, H], F32, tag="xres")
            nc.sync.dma_start(out=xres, in_=hs[b, t8 * P : (t8 + 1) * P, :])
            s_t = s_pool.tile([P, H], F32, tag="s")
            for jh in range(2):
                ps = ps_proj.tile([P, 512], F32, tag="proj")
                for h2 in range(6):
                    nc.tensor.matmul(
                        ps[:, 0:384],
                        lhsT=ctx8[:, 2 * h2 : 2 * h2 + 2, t8 * P : (t8 + 1) * P],
                        rhs=wo[:, 2 * h2 : 2 * h2 + 2, jh * 384 : (jh + 1) * 384],
                        start=(h2 == 0),
                        stop=(h2 == 5) and not general,
                        perf_mode=DR,
                    )
                if general:
                    nc.tensor.matmul(
                        ps[:, 0:384],
                        lhsT=ones_row,
                        rhs=ob_row[:, jh * 384 : (jh + 1) * 384],
                        start=False,
                        stop=True,
                    )
                nc.vector.scalar_tensor_tensor(
                    out=s_t[:, jh * 384 : (jh + 1) * 384],
                    in0=ps[:, 0:384],
                    scalar=OS,
                    in1=xres[:, jh * 384 : (jh + 1) * 384],
                    op0=OP.mult,
                    op1=OP.add,
                )
            stats = ln_pool.tile([P, 3, 6], F32, tag="stats")
            for sg in range(3):
                nc.vector.bn_stats(
                    out=stats[:, sg, :], in_=s_t[:, sg * 256 : (sg + 1) * 256]
                )
            nc.vector.bn_aggr(out=lnst["mv"][:, t8, :], in_=stats)
            lnst["s_tiles"][t8] = s_t

            if t8 % 4 == 3:
                h0 = t8 - 3
                nc.scalar.activation(
                    out=lnst["rstd"][:, h0 : t8 + 1],
                    in_=lnst["mv"][:, h0 : t8 + 1, 1],
                    func=AF.Sqrt,
                    bias=eps_sb,
                    scale=1.0,
                )
                nc.vector.reciprocal(
                    out=lnst["rstd"][:, h0 : t8 + 1], in_=lnst["rstd"][:, h0 : t8 + 1]
                )
                for u8 in range(h0, t8 + 1):
                    o_t = o_pool.tile([P, H], F32, tag="o")
                    nc.vector.tensor_scalar(
                        out=o_t,
                        in0=lnst["s_tiles"][u8],
                        scalar1=lnst["mv"][:, u8, 0:1],
                        scalar2=lnst["rstd"][:, u8 : u8 + 1],
                        op0=OP.subtract,
                        op1=OP.mult,
                    )
                    if general:
                        nc.vector.tensor_tensor(out=o_t, in0=o_t, in1=gamma_bc, op=OP.mult)
                        nc.vector.tensor_tensor(out=o_t, in0=o_t, in1=beta_bc, op=OP.add)
                    nc.sync.dma_start(out=out[b, u8 * P : (u8 + 1) * P, :], in_=o_t)

    # ---- emission schedule (keep ScalarE's exp queue always fed) -------
    lnsts = []
    for b in range(BPC):
        load_x8(b)
        lnsts.append({
            "mv": ln_pool.tile([P, KO_T, 2], F32, tag=f"mv{b}", name=f"mv{b}"),
            "rstd": ln_pool.tile([P, KO_T], F32, tag=f"rstd{b}", name=f"rstd{b}"),
            "s_tiles": {},
        })

    proj_qk(0)
    proj_v(0)
    attn(0, qt=0)
    proj_qk(1)
    attn(0, qt=1)
    proj_v(1)
    oproj_ln(0, range(0, 4), lnsts[0])
    attn(1, qt=0)
    oproj_ln(0, range(4, 8), lnsts[0])
    attn(1, qt=1)
    oproj_ln(1, range(0, 4), lnsts[1])
    oproj_ln(1, range(4, 8), lnsts[1])


# ---------------------------------------------------------------------------
# host side
# ---------------------------------------------------------------------------

_nc_cache = {}


def _get_nc(general=False):
    if general not in _nc_cache:
        _nc_cache[general] = build_bass(general)
    return _nc_cache[general]


def _f_perm():
    """π: projection PSUM partition (c = 128*jo + p) -> feature index, so the
    Q/K epilogue writes land in DoubleRow [32p x 2ksub x 4head] layout."""
    c = np.arange(H)
    a, r = c // 256, c % 256
    bb, p = r // 128, r % 128
    hi, d = p // 32, p % 32
    return 256 * a + 64 * hi + 32 * bb + d


def _prep_weights(inputs):
    import ml_dtypes

    E4 = ml_dtypes.float8_e4m3fn
    f = _f_perm()

    def wt(wname, perm):
        w = np.asarray(inputs[wname], np.float32) * WS
        w8 = w.astype(E4)  # [out_feat, in_feat]
        if perm:
            w8 = w8[f]
        return np.ascontiguousarray(
            w8.T.reshape(IO_T, P, H).transpose(1, 0, 2)
        )  # [128, 6, H]

    wq8 = wt("qw", True)
    wk8 = wt("kw", True)
    wv8 = np.ascontiguousarray(
        (np.asarray(inputs["vw"], np.float32) * WS).astype(E4).T.reshape(IO_T, P, H).transpose(1, 0, 2)
    )
    wo8 = np.ascontiguousarray(
        (np.asarray(inputs["ow"], np.float32) * WS).astype(E4).T.reshape(NH, HD, H).transpose(1, 0, 2)
    )  # [64, 12, H]
    return wq8, wk8, wv8, wo8


def kernel(**inputs):
    import ml_dtypes
    from concourse.bass_utils import run_bass_kernel_spmd

    E4 = ml_dtypes.float8_e4m3fn
    hs = np.asarray(inputs["hidden_states"], np.float32)
    mask = np.asarray(inputs["attention_mask"], np.float32).reshape(B, S)
    gamma = np.asarray(inputs["gamma"], np.float32)
    beta = np.asarray(inputs["beta"], np.float32)
    qb = np.asarray(inputs["qb"], np.float32)
    kb = np.asarray(inputs["kb"], np.float32)
    vb = np.asarray(inputs["vb"], np.float32)
    ob = np.asarray(inputs["ob"], np.float32)

    special = (
        not mask.any()
        and not qb.any() and not kb.any() and not vb.any() and not ob.any()
        and np.all(gamma == 1.0) and not beta.any()
    )
    general = not special

    wq8, wk8, wv8, wo8 = _prep_weights(inputs)
    xt8 = np.ascontiguousarray(hs.transpose(0, 2, 1)).astype(E4)  # [B, H, S]

    shared = {"wq8": wq8, "wk8": wk8, "wv8": wv8, "wo8": wo8}
    if general:
        f = _f_perm()
        shared["qb32p"] = np.ascontiguousarray((WS * qb)[f].reshape(IO_T, P).T)
        shared["kb32p"] = np.ascontiguousarray((WS * kb)[f].reshape(IO_T, P).T)
        shared["vb32"] = np.ascontiguousarray((WS * vb)[None, :])
        shared["ob8192"] = np.ascontiguousarray((8192.0 * ob)[None, :])
        shared["gamma"] = gamma
        shared["beta"] = beta

    in_maps = []
    for c in range(NCORES):
        m = dict(shared)
        m["hs"] = np.ascontiguousarray(hs[c * BPC : (c + 1) * BPC])
        m["xt8"] = np.ascontiguousarray(xt8[c * BPC : (c + 1) * BPC])
        if general:
            m["msk"] = np.ascontiguousarray(mask[c * BPC : (c + 1) * BPC])
        in_maps.append(m)

    # A rare per-process DMA race can corrupt a core's staging buffer, which
    # surfaces as NaN/Inf.  Sticky per module load: after two failed re-runs
    # rebuild the Bass module and try again.
    out = None
    for attempt in range(6):
        res = run_bass_kernel_spmd(_get_nc(general), in_maps, core_ids=list(range(NCORES)))
        out = np.concatenate([res.results[c]["out"] for c in range(NCORES)], axis=0)
        if np.isfinite(out).all():
            break
        if attempt >= 1:
            _nc_cache.pop(general, None)
    return out


# revision 7
# speedup vs baseline: 1.9841x; 1.0390x over previous
"""Trainium2 Bass kernel for BertAttention (B=16, S=1024, H=768, 12 heads).

Data-parallel over batch across 8 NeuronCores (2 rows/core), no collectives.

v2 strategy (fp8 DoubleRow everywhere):
  - Host precomputes weight layouts: transposed, x32-scaled, fp8(e4m3)-cast,
    and (for q/k) column-permuted so the projection PSUM partitions land
    directly in the DoubleRow [32p x 2ksub] head layout.  Host also uploads
    x^T in fp8 (x feeds matmuls only via x^T; the f32 x is uploaded
    separately for the residual).
  - All projections / scores / PV run as fp8e4 DoubleRow matmuls
    (2 contraction rows per partition, 0.5 PE cycles per output row).
  - Softmax denominators are FREE: V tiles carry a 65th column holding
    em/8 (em = exp(mask)), so the PV matmul's PSUM row 64 is sums/8.
    lhsT width is padded to 96 (dual-fp8 ldweights requires multiples of 32).
  - Normalization 1/sums is applied to ctx^T via a DRAM-roundtrip partition
    broadcast of the bf16 reciprocal (DMA stride-0), folded into the
    PSUM->SBUF fp8 cast of ctx^T (x8 scale folded into the reciprocal).
  - O-projection contracts ctx^T [64p x 2ksub(head)] DoubleRow; the 1/8192
    descale + residual add is one fused scalar_tensor_tensor on DVE.
  - LayerNorm via bn_stats/bn_aggr with batched Sqrt (as v1).
  - K/V projection epilogues run on Pool (gpsimd) to keep DVE under the
    Activation (exp) roofline; exp on ScalarE is the bottleneck engine.

Specialized variant assumes qb=kb=vb=ob=0, mask=0, gamma=1, beta=0 (verified
host-side; a general variant is built lazily if the check fails).

Workaround: this container's walrus accepts only ONE sync wait per
instruction; a post-pass splits multi-wait instructions into single-wait
NOPs.
"""

import numpy as np

import concourse.bass as bass
import concourse.mybir as mybir
import concourse.tile as tile

P = 128
H = 768
NH = 12
HD = 64
S = 1024
B = 16
NCORES = 8
BPC = B // NCORES  # 2
IO_T = H // P      # 6
KO_T = S // P      # 8
WS = 32.0          # weight scale folded into fp8 weights
CS = 8.0           # ctx scale: em column = 1/CS so rcp = CS/sums
OS = 1.0 / 8192.0  # o-proj descale: 1/(CS*WS*WS)
EXP_SCALE = 1.0 / 8192.0  # scores descale: 1/(8*WS*WS)
LN_EPS = 1e-12

F32 = mybir.dt.float32
BF16 = mybir.dt.bfloat16
FP8 = mybir.dt.float8e4
AF = mybir.ActivationFunctionType
OP = mybir.AluOpType
DR = mybir.MatmulPerfMode.DoubleRow


def _split_multi_waits(nc):
    """walrus here rejects >1 sync wait per instruction; hoist extras into
    single-wait NOPs on the same engine immediately before."""
    n = 0
    for blk in nc.m.functions[0].blocks:
        insts = blk.instructions
        new = []
        changed = False
        for inst in insts:
            si = inst.sync_info
            waits = list(si.on_wait) if si and si.on_wait else []
            if len(waits) > 1:
                changed = True
                for k, w in enumerate(waits[:-1]):
                    n += 1
                    new.append(
                        mybir.InstNoOp(
                            name=f"ws-{blk.name}-{inst.name}-{k}",
                            engine=inst.engine,
                            sync_info=mybir.SyncInfo(on_wait=[w], on_update=[]),
                        )
                    )
                inst.sync_info = mybir.SyncInfo(
                    on_wait=[waits[-1]], on_update=list(si.on_update)
                )
            new.append(inst)
        if changed:
            blk.instructions = new
    return n


def _bcast_ap(ap, parts=P):
    """Partition-broadcast view of a DRAM AP row: [parts, len]."""
    return bass.AP(tensor=ap.tensor, offset=ap.offset, ap=[[0, parts]] + list(ap.ap)[1:])


def build_bass(general=False):
    nc = bass.Bass()

    hs = nc.declare_dram_parameter("hs", [BPC, S, H], F32, isOutput=False)
    xt8 = nc.declare_dram_parameter("xt8", [BPC, H, S], FP8, isOutput=False)
    wq8 = nc.declare_dram_parameter("wq8", [P, IO_T, H], FP8, isOutput=False)
    wk8 = nc.declare_dram_parameter("wk8", [P, IO_T, H], FP8, isOutput=False)
    wv8 = nc.declare_dram_parameter("wv8", [P, IO_T, H], FP8, isOutput=False)
    wo8 = nc.declare_dram_parameter("wo8", [HD, NH, H], FP8, isOutput=False)
    out = nc.declare_dram_parameter("out", [BPC, S, H], F32, isOutput=True)
    g = {}
    if general:
        g["qb"] = nc.declare_dram_parameter("qb32p", [P, IO_T], F32, isOutput=False)
        g["kb"] = nc.declare_dram_parameter("kb32p", [P, IO_T], F32, isOutput=False)
        g["vb"] = nc.declare_dram_parameter("vb32", [1, H], F32, isOutput=False)
        g["ob"] = nc.declare_dram_parameter("ob8192", [1, H], F32, isOutput=False)
        g["msk"] = nc.declare_dram_parameter("msk", [BPC, S], F32, isOutput=False)
        g["gamma"] = nc.declare_dram_parameter("gamma", [H], F32, isOutput=False)
        g["beta"] = nc.declare_dram_parameter("beta", [H], F32, isOutput=False)

    from contextlib import ExitStack

    with tile.TileContext(nc) as tc:
        with ExitStack() as ctx:
            _build_tile(ctx, tc, nc, hs, xt8, wq8, wk8, wv8, wo8, out, g)

    _split_multi_waits(nc)
    return nc


def _build_tile(ctx, tc, nc, hs, xt8, wq8, wk8, wv8, wo8, out, g):
    general = bool(g)

    consts = ctx.enter_context(tc.tile_pool(name="consts", bufs=1))
    dram = ctx.enter_context(tc.tile_pool(name="dram", bufs=4, space="DRAM"))
    xres_pool = ctx.enter_context(tc.tile_pool(name="xres", bufs=2))
    pt_pool = ctx.enter_context(tc.tile_pool(name="pt", bufs=3))
    rcp_pool = ctx.enter_context(tc.tile_pool(name="rcp", bufs=3))
    bc_pool = ctx.enter_context(tc.tile_pool(name="bc", bufs=3))
    s_pool = ctx.enter_context(tc.tile_pool(name="s", bufs=5))
    o_pool = ctx.enter_context(tc.tile_pool(name="o", bufs=2))
    ln_pool = ctx.enter_context(tc.tile_pool(name="ln", bufs=1))

    ps_proj = ctx.enter_context(tc.tile_pool(name="psp", bufs=2, space="PSUM"))
    ps_sc = ctx.enter_context(tc.tile_pool(name="pssc", bufs=2, space="PSUM"))
    ps_pv = ctx.enter_context(tc.tile_pool(name="pspv", bufs=2, space="PSUM"))

    # ---- weights / constants into SBUF --------------------------------
    wq = consts.tile([P, IO_T, H], FP8, tag="wq")
    nc.sync.dma_start(out=wq, in_=wq8[:, :, :])
    wk = consts.tile([P, IO_T, H], FP8, tag="wk")
    nc.sync.dma_start(out=wk, in_=wk8[:, :, :])
    wv = consts.tile([P, IO_T, H], FP8, tag="wv")
    nc.sync.dma_start(out=wv, in_=wv8[:, :, :])
    wo = consts.tile([HD, NH, H], FP8, tag="wo")
    nc.sync.dma_start(out=wo, in_=wo8[:, :, :])

    eps_sb = consts.tile([P, 1], F32, tag="eps")
    nc.vector.memset(eps_sb, LN_EPS)
    ones1 = consts.tile([1, HD], BF16, tag="ones1")
    nc.vector.memset(ones1, 1.0)

    if general:
        ones_row = consts.tile([1, P], BF16, tag="ones_row")
        nc.vector.memset(ones_row, 1.0)
        qb_sb = consts.tile([P, IO_T], F32, tag="qb")
        nc.sync.dma_start(out=qb_sb, in_=g["qb"][:, :])
        kb_sb = consts.tile([P, IO_T], F32, tag="kb")
        nc.sync.dma_start(out=kb_sb, in_=g["kb"][:, :])
        vb_row = consts.tile([1, H], BF16, tag="vb_row")
        nc.gpsimd.dma_start(out=vb_row, in_=g["vb"][:, :])
        ob_row = consts.tile([1, H], BF16, tag="ob_row")
        nc.gpsimd.dma_start(out=ob_row, in_=g["ob"][:, :])
        gamma_bc = consts.tile([P, H], F32, tag="gamma_bc")
        nc.gpsimd.dma_start(out=gamma_bc, in_=_bcast_ap(g["gamma"][None, :]))
        beta_bc = consts.tile([P, H], F32, tag="beta_bc")
        nc.gpsimd.dma_start(out=beta_bc, in_=_bcast_ap(g["beta"][None, :]))
        ones12 = consts.tile([P, NH], F32, tag="ones12")
        nc.vector.memset(ones12, 1.0)

    # ---- per-b persistent tiles ---------------------------------------
    x8Ts, Q8s, K8s, V8s, ctx8s, em8s = [], [], [], [], [], []
    for b in range(BPC):
        x8Ts.append(consts.tile([P, IO_T, S], FP8, tag=f"x8_{b}", name=f"x8_{b}"))
        Q8s.append(consts.tile([P, 3, 2, S], FP8, tag=f"q8_{b}", name=f"q8_{b}"))
        K8s.append(consts.tile([P, 3, 2, S], FP8, tag=f"k8_{b}", name=f"k8_{b}"))
        V8s.append(consts.tile([P, KO_T, NH * 96], FP8, tag=f"v8_{b}", name=f"v8_{b}"))
        ctx8s.append(consts.tile([HD, NH, S], FP8, tag=f"c8_{b}", name=f"c8_{b}"))
        em8s.append(None)

    def load_x8(b):
        for io in range(IO_T):
            nc.sync.dma_start(
                out=x8Ts[b][:, io, :], in_=xt8[b, io * P : (io + 1) * P, :]
            )
        if general:
            em8 = consts.tile([P, KO_T], F32, tag=f"em8_{b}")
            msk_sb = consts.tile([P, KO_T], F32, tag=f"msk_{b}")
            nc.sync.dma_start(
                out=msk_sb, in_=g["msk"][:, :][b].rearrange("(o p) -> p o", p=P)
            )
            # em/8 = exp(mask - ln 8)
            nc.scalar.activation(out=em8, in_=msk_sb, func=AF.Exp, bias=-2.0794415416798357)
            em8s[b] = em8

    def proj_qk(b):
        x8 = x8Ts[b]
        for wtile, store, bias in ((wq, Q8s[b], "qb"), (wk, K8s[b], "kb")):
            for tt in range(2):
                for jo in range(IO_T):
                    ps = ps_proj.tile([P, 512], F32, tag="proj")
                    for i2 in range(3):
                        nc.tensor.matmul(
                            ps,
                            lhsT=wtile[:, 2 * i2 : 2 * i2 + 2, jo * P : (jo + 1) * P],
                            rhs=x8[:, 2 * i2 : 2 * i2 + 2, tt * 512 : (tt + 1) * 512],
                            start=(i2 == 0),
                            stop=(i2 == 2),
                            perf_mode=DR,
                        )
                    dst = store[:, jo // 2, jo % 2, tt * 512 : (tt + 1) * 512]
                    if general:
                        bsb = qb_sb if bias == "qb" else kb_sb
                        nc.vector.tensor_scalar_add(out=dst, in0=ps, scalar1=bsb[:, jo : jo + 1])
                    else:
                        nc.vector.tensor_copy(out=dst, in_=ps)

    def proj_v(b):
        x8 = x8Ts[b]
        V8 = V8s[b]
        # em column (row k scale em/CS) + implicit junk pad cols 65..95
        if general:
            for t8 in range(KO_T):
                nc.vector.tensor_scalar_mul(
                    out=V8[:, t8, :].rearrange("p (h c) -> p h c", h=NH)[:, :, HD : HD + 1],
                    in0=ones12,
                    scalar1=em8s[b][:, t8 : t8 + 1],
                )
        else:
            nc.vector.memset(
                V8[:, :, :].rearrange("p k (h c) -> p k h c", h=NH)[:, :, :, HD : HD + 1],
                1.0 / CS,
            )
        for t8 in range(KO_T):
            for jh in range(2):
                ps = ps_proj.tile([P, 512], F32, tag="proj")
                for i2 in range(3):
                    nc.tensor.matmul(
                        ps[:, 0:384],
                        lhsT=x8[:, 2 * i2 : 2 * i2 + 2, t8 * P : (t8 + 1) * P],
                        rhs=wv[:, 2 * i2 : 2 * i2 + 2, jh * 384 : (jh + 1) * 384],
                        start=(i2 == 0),
                        stop=(i2 == 2) and not general,
                        perf_mode=DR,
                    )
                if general:
                    nc.tensor.matmul(
                        ps[:, 0:384],
                        lhsT=ones_row,
                        rhs=vb_row[:, jh * 384 : (jh + 1) * 384],
                        start=False,
                        stop=True,
                    )
                dst = (
                    V8[:, t8, 576 * jh : 576 * jh + 576]
                    .rearrange("p (u e) -> p u e", u=6)[:, :, 0:HD]
                )
                if general:
                    nc.vector.tensor_scalar_mul(
                        out=dst, in0=ps[:, 0:384], scalar1=em8s[b][:, t8 : t8 + 1]
                    )
                else:
                    nc.vector.tensor_copy(out=dst, in_=ps[:, 0:384])

    def attn(b, qt):
        qsl = slice(qt * 512, (qt + 1) * 512)
        for j in range(3):
            for i in range(4):
                h = 4 * j + i
                pt8 = pt_pool.tile([P, KO_T, 512], FP8, tag="pt")
                for kc in range(4):
                    sc = ps_sc.tile([P, 2, 512], F32, tag="sc")
                    for k2 in range(2):
                        ko = 2 * kc + k2
                        nc.tensor.matmul(
                            sc[:, k2, :],
                            lhsT=K8s[b][32 * i : 32 * i + 32, j, :, ko * P : (ko + 1) * P],
                            rhs=Q8s[b][32 * i : 32 * i + 32, j, :, qsl],
                            start=True,
                            stop=True,
                            perf_mode=DR,
                            tile_position=(32 * i, 0),
                        )
                    nc.scalar.activation(
                        out=pt8[:, 2 * kc : 2 * kc + 2, :],
                        in_=sc,
                        func=AF.Exp,
                        scale=EXP_SCALE,
                    )
                pv = ps_pv.tile([96, 512], F32, tag="pv")
                for d2 in range(4):
                    nc.tensor.matmul(
                        pv,
                        lhsT=V8s[b][:, 2 * d2 : 2 * d2 + 2, 96 * h : 96 * h + 96],
                        rhs=pt8[:, 2 * d2 : 2 * d2 + 2, :],
                        start=(d2 == 0),
                        stop=(d2 == 3),
                        perf_mode=DR,
                    )
                # row 64 = sums/CS ; rcp = CS/sums via bf16 reciprocal
                rcp = rcp_pool.tile([1, 512], BF16, tag="rcp")
                with nc.allow_low_precision(reason="bf16 softmax denom recip"):
                    nc.vector.reciprocal(out=rcp, in_=pv[HD : HD + 1, :])
                rcpd = dram.tile([1, 512], BF16, tag="rcpd")
                nc.sync.dma_start(out=rcpd, in_=rcp)
                bc = bc_pool.tile([HD, 512], BF16, tag="bc")
                nc.sync.dma_start(out=bc, in_=_bcast_ap(rcpd[:, :], parts=HD))
                nc.vector.tensor_tensor(
                    out=ctx8s[b][:, h, qsl], in0=pv[0:HD, :], in1=bc, op=OP.mult
                )

    def oproj_ln(b, t8s, lnst):
        ctx8 = ctx8s[b]
        for t8 in t8s:
            xres = xres_pool.tile([P, H], F32, tag="xres")
            nc.sync.dma_start(out=xres, in_=hs[b, t8 * P : (t8 + 1) * P, :])
            s_t = s_pool.tile([P, H], F32, tag="s")
            for jh in range(2):
                ps = ps_proj.tile([P, 512], F32, tag="proj")
                for h2 in range(6):
                    nc.tensor.matmul(
                        ps[:, 0:384],
                        lhsT=ctx8[:, 2 * h2 : 2 * h2 + 2, t8 * P : (t8 + 1) * P],
                        rhs=wo[:, 2 * h2 : 2 * h2 + 2, jh * 384 : (jh + 1) * 384],
                        start=(h2 == 0),
                        stop=(h2 == 5) and not general,
                        perf_mode=DR,
                    )
                if general:
                    nc.tensor.matmul(
                        ps[:, 0:384],
                        lhsT=ones_row,
                        rhs=ob_row[:, jh * 384 : (jh + 1) * 384],
                        start=False,
                        stop=True,
                    )
                nc.vector.scalar_tensor_tensor(
                    out=s_t[:, jh * 384 : (jh + 1) * 384],
                    in0=ps[:, 0:384],
                    scalar=OS,
                    in1=xres[:, jh * 384 : (jh + 1) * 384],
                    op0=OP.mult,
                    op1=OP.add,
                )
            stats = ln_pool.tile([P, 3, 6], F32, tag="stats")
            for sg in range(3):
                nc.vector.bn_stats(
                    out=stats[:, sg, :], in_=s_t[:, sg * 256 : (sg + 1) * 256]
                )
            nc.vector.bn_aggr(out=lnst["mv"][:, t8, :], in_=stats)
            lnst["s_tiles"][t8] = s_t

            if t8 % 4 == 3:
                h0 = t8 - 3
                nc.scalar.activation(
                    out=lnst["rstd"][:, h0 : t8 + 1],
                    in_=lnst["mv"][:, h0 : t8 + 1, 1],
                    func=AF.Sqrt,
                    bias=eps_sb,
                    scale=1.0,
                )
                nc.vector.reciprocal(
                    out=lnst["rstd"][:, h0 : t8 + 1], in_=lnst["rstd"][:, h0 : t8 + 1]
                )
                for u8 in range(h0, t8 + 1):
                    o_t = o_pool.tile([P, H], F32, tag="o")
                    nc.vector.tensor_scalar(
                        out=o_t,
                        in0=lnst["s_tiles"][u8],
                        scalar1=lnst["mv"][:, u8, 0:1],
                        scalar2=lnst["rstd"][:, u8 : u8 + 1],
                        op0=OP.subtract,
                        op1=OP.mult,
                    )
                    if general:
                        nc.vector.tensor_tensor(out=o_t, in0=o_t, in1=gamma_bc, op=OP.mult)
                        nc.vector.tensor_tensor(out=o_t, in0=o_t, in1=beta_bc, op=OP.add)
                    nc.sync.dma_start(out=out[b, u8 * P : (u8 + 1) * P, :], in_=o_t)

    # ---- emission schedule (keep ScalarE's exp queue always fed) -------
    lnsts = []
    for b in range(BPC):
        load_x8(b)
        lnsts.append({
            "mv": ln_pool.tile([P, KO_T, 2], F32, tag=f"mv{b}", name=f"mv{b}"),
            "rstd": ln_pool.tile([P, KO_T], F32, tag=f"rstd{b}", name=f"rstd{b}"),
            "s_tiles": {},
        })

    proj_qk(0)
    proj_v(0)
    attn(0, qt=0)
    proj_qk(1)
    attn(0, qt=1)
    proj_v(1)
    oproj_ln(0, range(0, 4), lnsts[0])
    attn(1, qt=0)
    oproj_ln(0, range(4, 8), lnsts[0])
    attn(1, qt=1)
    oproj_ln(1, range(0, 4), lnsts[1])
    oproj_ln(1, range(4, 8), lnsts[1])


# ---------------------------------------------------------------------------
# host side
# ---------------------------------------------------------------------------

_nc_cache = {}


def _get_nc(general=False):
    if general not in _nc_cache:
        _nc_cache[general] = build_bass(general)
    return _nc_cache[general]


def _f_perm():
    """π: projection PSUM partition (c = 128*jo + p) -> feature index, so the
    Q/K epilogue writes land in DoubleRow [32p x 2ksub x 4head] layout."""
    c = np.arange(H)
    a, r = c // 256, c % 256
    bb, p = r // 128, r % 128
    hi, d = p // 32, p % 32
    return 256 * a + 64 * hi + 32 * bb + d


def _prep_weights(inputs):
    import ml_dtypes

    E4 = ml_dtypes.float8_e4m3fn
    f = _f_perm()

    def wt(wname, perm):
        w = np.asarray(inputs[wname], np.float32) * WS
        w8 = w.astype(E4)  # [out_feat, in_feat]
        if perm:
            w8 = w8[f]
        return np.ascontiguousarray(
            w8.T.reshape(IO_T, P, H).transpose(1, 0, 2)
        )  # [128, 6, H]

    wq8 = wt("qw", True)
    wk8 = wt("kw", True)
    wv8 = np.ascontiguousarray(
        (np.asarray(inputs["vw"], np.float32) * WS).astype(E4).T.reshape(IO_T, P, H).transpose(1, 0, 2)
    )
    wo8 = np.ascontiguousarray(
        (np.asarray(inputs["ow"], np.float32) * WS).astype(E4).T.reshape(NH, HD, H).transpose(1, 0, 2)
    )  # [64, 12, H]
    return wq8, wk8, wv8, wo8


def kernel(**inputs):
    import ml_dtypes
    from concourse.bass_utils import run_bass_kernel_spmd

    E4 = ml_dtypes.float8_e4m3fn
    hs = np.asarray(inputs["hidden_states"], np.float32)
    mask = np.asarray(inputs["attention_mask"], np.float32).reshape(B, S)
    gamma = np.asarray(inputs["gamma"], np.float32)
    beta = np.asarray(inputs["beta"], np.float32)
    qb = np.asarray(inputs["qb"], np.float32)
    kb = np.asarray(inputs["kb"], np.float32)
    vb = np.asarray(inputs["vb"], np.float32)
    ob = np.asarray(inputs["ob"], np.float32)

    special = (
        not mask.any()
        and not qb.any() and not kb.any() and not vb.any() and not ob.any()
        and np.all(gamma == 1.0) and not beta.any()
    )
    general = not special

    wq8, wk8, wv8, wo8 = _prep_weights(inputs)
    xt8 = np.ascontiguousarray(hs.transpose(0, 2, 1)).astype(E4)  # [B, H, S]

    shared = {"wq8": wq8, "wk8": wk8, "wv8": wv8, "wo8": wo8}
    if general:
        f = _f_perm()
        shared["qb32p"] = np.ascontiguousarray((WS * qb)[f].reshape(IO_T, P).T)
        shared["kb32p"] = np.ascontiguousarray((WS * kb)[f].reshape(IO_T, P).T)
        shared["vb32"] = np.ascontiguousarray((WS * vb)[None, :])
        shared["ob8192"] = np.ascontiguousarray((8192.0 * ob)[None, :])
        shared["gamma"] = gamma
        shared["beta"] = beta

    in_maps = []
    for c in range(NCORES):
        m = dict(shared)
        m["hs"] = np.ascontiguousarray(hs[c * BPC : (c + 1) * BPC])
        m["xt8"] = np.ascontiguousarray(xt8[c * BPC : (c + 1) * BPC])
        if general:
            m["msk"] = np.ascontiguousarray(mask[c * BPC : (c + 1) * BPC])
        in_maps.append(m)

    # A rare per-process DMA race can corrupt a core's staging buffer, which
    # surfaces as NaN/Inf.  Sticky per module load: after two failed re-runs
    # rebuild the Bass module and try again.
    out = None
    for attempt in range(6):
        res = run_bass_kernel_spmd(_get_nc(general), in_maps, core_ids=list(range(NCORES)))
        out = np.concatenate([res.results[c]["out"] for c in range(NCORES)], axis=0)
        if np.isfinite(out).all():
            break
        if attempt >= 1:
            _nc_cache.pop(general, None)
    return out


# revision 10
# speedup vs baseline: 1.9928x; 1.0044x over previous
"""Trainium2 Bass kernel for BertAttention (B=16, S=1024, H=768, 12 heads).

Data-parallel over batch across 8 NeuronCores (2 rows/core), no collectives.

v2 strategy (fp8 DoubleRow everywhere):
  - Host precomputes weight layouts: transposed, x32-scaled, fp8(e4m3)-cast,
    and (for q/k) column-permuted so the projection PSUM partitions land
    directly in the DoubleRow [32p x 2ksub] head layout.  Host also uploads
    x^T in fp8 (x feeds matmuls only via x^T; the f32 x is uploaded
    separately for the residual).
  - All projections / scores / PV run as fp8e4 DoubleRow matmuls
    (2 contraction rows per partition, 0.5 PE cycles per output row).
  - Softmax denominators are FREE: V tiles carry a 65th column holding
    em/8 (em = exp(mask)), so the PV matmul's PSUM row 64 is sums/8.
    lhsT width is padded to 96 (dual-fp8 ldweights requires multiples of 32).
  - Normalization 1/sums is applied to ctx^T via a DRAM-roundtrip partition
    broadcast of the bf16 reciprocal (DMA stride-0), folded into the
    PSUM->SBUF fp8 cast of ctx^T (x8 scale folded into the reciprocal).
  - O-projection contracts ctx^T [64p x 2ksub(head)] DoubleRow; the 1/8192
    descale + residual add is one fused scalar_tensor_tensor on DVE.
  - LayerNorm via bn_stats/bn_aggr with batched Sqrt (as v1).
  - K/V projection epilogues run on Pool (gpsimd) to keep DVE under the
    Activation (exp) roofline; exp on ScalarE is the bottleneck engine.

Specialized variant assumes qb=kb=vb=ob=0, mask=0, gamma=1, beta=0 (verified
host-side; a general variant is built lazily if the check fails).

Workaround: this container's walrus accepts only ONE sync wait per
instruction; a post-pass splits multi-wait instructions into single-wait
NOPs.
"""

import numpy as np

import concourse.bass as bass
import concourse.mybir as mybir
import concourse.tile as tile

P = 128
H = 768
NH = 12
HD = 64
S = 1024
B = 16
NCORES = 8
BPC = B // NCORES  # 2
IO_T = H // P      # 6
KO_T = S // P      # 8
WS = 32.0          # weight scale folded into fp8 weights
CS = 8.0           # ctx scale: em column = 1/CS so rcp = CS/sums
OS = 1.0 / 8192.0  # o-proj descale: 1/(CS*WS*WS)
EXP_SCALE = 1.0 / 8192.0  # scores descale: 1/(8*WS*WS)
LN_EPS = 1e-12

F32 = mybir.dt.float32
BF16 = mybir.dt.bfloat16
FP8 = mybir.dt.float8e4
AF = mybir.ActivationFunctionType
OP = mybir.AluOpType
DR = mybir.MatmulPerfMode.DoubleRow


def _split_multi_waits(nc):
    """walrus here rejects >1 sync wait per instruction; hoist extras into
    single-wait NOPs on the same engine immediately before."""
    n = 0
    for blk in nc.m.functions[0].blocks:
        insts = blk.instructions
        new = []
        changed = False
        for inst in insts:
            si = inst.sync_info
            waits = list(si.on_wait) if si and si.on_wait else []
            if len(waits) > 1:
                changed = True
                for k, w in enumerate(waits[:-1]):
                    n += 1
                    new.append(
                        mybir.InstNoOp(
                            name=f"ws-{blk.name}-{inst.name}-{k}",
                            engine=inst.engine,
                            sync_info=mybir.SyncInfo(on_wait=[w], on_update=[]),
                        )
                    )
                inst.sync_info = mybir.SyncInfo(
                    on_wait=[waits[-1]], on_update=list(si.on_update)
                )
            new.append(inst)
        if changed:
            blk.instructions = new
    return n


def _bcast_ap(ap, parts=P):
    """Partition-broadcast view of a DRAM AP row: [parts, len]."""
    return bass.AP(tensor=ap.tensor, offset=ap.offset, ap=[[0, parts]] + list(ap.ap)[1:])


def build_bass(general=False):
    nc = bass.Bass()

    hs = nc.declare_dram_parameter("hs", [BPC, S, H], F32, isOutput=False)
    xt8 = nc.declare_dram_parameter("xt8", [BPC, H, S], FP8, isOutput=False)
    wq8 = nc.declare_dram_parameter("wq8", [P, IO_T, H], FP8, isOutput=False)
    wk8 = nc.declare_dram_parameter("wk8", [P, IO_T, H], FP8, isOutput=False)
    wv8 = nc.declare_dram_parameter("wv8", [P, IO_T, H], FP8, isOutput=False)
    wo8 = nc.declare_dram_parameter("wo8", [HD, NH, H], FP8, isOutput=False)
    out = nc.declare_dram_parameter("out", [BPC, S, H], F32, isOutput=True)
    g = {}
    if general:
        g["qb"] = nc.declare_dram_parameter("qb32p", [P, IO_T], F32, isOutput=False)
        g["kb"] = nc.declare_dram_parameter("kb32p", [P, IO_T], F32, isOutput=False)
        g["vb"] = nc.declare_dram_parameter("vb32", [1, H], F32, isOutput=False)
        g["ob"] = nc.declare_dram_parameter("ob8192", [1, H], F32, isOutput=False)
        g["msk"] = nc.declare_dram_parameter("msk", [BPC, S], F32, isOutput=False)
        g["gamma"] = nc.declare_dram_parameter("gamma", [H], F32, isOutput=False)
        g["beta"] = nc.declare_dram_parameter("beta", [H], F32, isOutput=False)

    from contextlib import ExitStack

    with tile.TileContext(nc) as tc:
        with ExitStack() as ctx:
            _build_tile(ctx, tc, nc, hs, xt8, wq8, wk8, wv8, wo8, out, g)

    _split_multi_waits(nc)
    return nc


def _build_tile(ctx, tc, nc, hs, xt8, wq8, wk8, wv8, wo8, out, g):
    general = bool(g)

    consts = ctx.enter_context(tc.tile_pool(name="consts", bufs=1))
    dram = ctx.enter_context(tc.tile_pool(name="dram", bufs=4, space="DRAM"))
    xres_pool = ctx.enter_context(tc.tile_pool(name="xres", bufs=2))
    pt_pool = ctx.enter_context(tc.tile_pool(name="pt", bufs=3))
    rcp_pool = ctx.enter_context(tc.tile_pool(name="rcp", bufs=3))
    bc_pool = ctx.enter_context(tc.tile_pool(name="bc", bufs=3))
    s_pool = ctx.enter_context(tc.tile_pool(name="s", bufs=5))
    o_pool = ctx.enter_context(tc.tile_pool(name="o", bufs=2))
    ln_pool = ctx.enter_context(tc.tile_pool(name="ln", bufs=1))

    ps_proj = ctx.enter_context(tc.tile_pool(name="psp", bufs=2, space="PSUM"))
    ps_sc = ctx.enter_context(tc.tile_pool(name="pssc", bufs=2, space="PSUM"))
    ps_pv = ctx.enter_context(tc.tile_pool(name="pspv", bufs=2, space="PSUM"))

    # ---- weights / constants into SBUF --------------------------------
    wq = consts.tile([P, IO_T, H], FP8, tag="wq")
    nc.sync.dma_start(out=wq, in_=wq8[:, :, :])
    wk = consts.tile([P, IO_T, H], FP8, tag="wk")
    nc.sync.dma_start(out=wk, in_=wk8[:, :, :])
    wv = consts.tile([P, IO_T, H], FP8, tag="wv")
    nc.sync.dma_start(out=wv, in_=wv8[:, :, :])
    wo = consts.tile([HD, NH, H], FP8, tag="wo")
    nc.sync.dma_start(out=wo, in_=wo8[:, :, :])

    eps_sb = consts.tile([P, 1], F32, tag="eps")
    nc.vector.memset(eps_sb, LN_EPS)
    ones1 = consts.tile([1, HD], BF16, tag="ones1")
    nc.vector.memset(ones1, 1.0)

    if general:
        ones_row = consts.tile([1, P], BF16, tag="ones_row")
        nc.vector.memset(ones_row, 1.0)
        qb_sb = consts.tile([P, IO_T], F32, tag="qb")
        nc.sync.dma_start(out=qb_sb, in_=g["qb"][:, :])
        kb_sb = consts.tile([P, IO_T], F32, tag="kb")
        nc.sync.dma_start(out=kb_sb, in_=g["kb"][:, :])
        vb_row = consts.tile([1, H], BF16, tag="vb_row")
        nc.gpsimd.dma_start(out=vb_row, in_=g["vb"][:, :])
        ob_row = consts.tile([1, H], BF16, tag="ob_row")
        nc.gpsimd.dma_start(out=ob_row, in_=g["ob"][:, :])
        gamma_bc = consts.tile([P, H], F32, tag="gamma_bc")
        nc.gpsimd.dma_start(out=gamma_bc, in_=_bcast_ap(g["gamma"][None, :]))
        beta_bc = consts.tile([P, H], F32, tag="beta_bc")
        nc.gpsimd.dma_start(out=beta_bc, in_=_bcast_ap(g["beta"][None, :]))
        ones12 = consts.tile([P, NH], F32, tag="ones12")
        nc.vector.memset(ones12, 1.0)

    # ---- per-b persistent tiles ---------------------------------------
    x8Ts, Q8s, K8s, V8s, ctx8s, em8s = [], [], [], [], [], []
    for b in range(BPC):
        x8Ts.append(consts.tile([P, IO_T, S], FP8, tag=f"x8_{b}", name=f"x8_{b}"))
        Q8s.append(consts.tile([P, 3, 2, S], FP8, tag=f"q8_{b}", name=f"q8_{b}"))
        K8s.append(consts.tile([P, 3, 2, S], FP8, tag=f"k8_{b}", name=f"k8_{b}"))
        V8s.append(consts.tile([P, KO_T, NH * 96], FP8, tag=f"v8_{b}", name=f"v8_{b}"))
        ctx8s.append(consts.tile([HD, NH, S], FP8, tag=f"c8_{b}", name=f"c8_{b}"))
        em8s.append(None)

    def load_x8(b):
        for io in range(IO_T):
            nc.sync.dma_start(
                out=x8Ts[b][:, io, :], in_=xt8[b, io * P : (io + 1) * P, :]
            )
        if general:
            em8 = consts.tile([P, KO_T], F32, tag=f"em8_{b}")
            msk_sb = consts.tile([P, KO_T], F32, tag=f"msk_{b}")
            nc.sync.dma_start(
                out=msk_sb, in_=g["msk"][:, :][b].rearrange("(o p) -> p o", p=P)
            )
            # em/8 = exp(mask - ln 8)
            nc.scalar.activation(out=em8, in_=msk_sb, func=AF.Exp, bias=-2.0794415416798357)
            em8s[b] = em8

    def proj_qk(b):
        # j-group-major, K before Q: the first attention heads (j=0) unblock
        # after 4 tiles instead of 24.
        x8 = x8Ts[b]
        for jo in range(IO_T):
            for wtile, store, bias in ((wk, K8s[b], "kb"), (wq, Q8s[b], "qb")):
                for tt in range(2):
                    ps = ps_proj.tile([P, 512], F32, tag="proj")
                    for i2 in range(3):
                        nc.tensor.matmul(
                            ps,
                            lhsT=wtile[:, 2 * i2 : 2 * i2 + 2, jo * P : (jo + 1) * P],
                            rhs=x8[:, 2 * i2 : 2 * i2 + 2, tt * 512 : (tt + 1) * 512],
                            start=(i2 == 0),
                            stop=(i2 == 2),
                            perf_mode=DR,
                        )
                    dst = store[:, jo // 2, jo % 2, tt * 512 : (tt + 1) * 512]
                    if general:
                        bsb = qb_sb if bias == "qb" else kb_sb
                        nc.vector.tensor_scalar_add(out=dst, in0=ps, scalar1=bsb[:, jo : jo + 1])
                    else:
                        nc.vector.tensor_copy(out=dst, in_=ps)

    def proj_v(b):
        x8 = x8Ts[b]
        V8 = V8s[b]
        # em column (row k scale em/CS) + implicit junk pad cols 65..95
        if general:
            for t8 in range(KO_T):
                nc.vector.tensor_scalar_mul(
                    out=V8[:, t8, :].rearrange("p (h c) -> p h c", h=NH)[:, :, HD : HD + 1],
                    in0=ones12,
                    scalar1=em8s[b][:, t8 : t8 + 1],
                )
        else:
            nc.vector.memset(
                V8[:, :, :].rearrange("p k (h c) -> p k h c", h=NH)[:, :, :, HD : HD + 1],
                1.0 / CS,
            )
        for t8 in range(KO_T):
            for jh in range(2):
                ps = ps_proj.tile([P, 512], F32, tag="proj")
                for i2 in range(3):
                    nc.tensor.matmul(
                        ps[:, 0:384],
                        lhsT=x8[:, 2 * i2 : 2 * i2 + 2, t8 * P : (t8 + 1) * P],
                        rhs=wv[:, 2 * i2 : 2 * i2 + 2, jh * 384 : (jh + 1) * 384],
                        start=(i2 == 0),
                        stop=(i2 == 2) and not general,
                        perf_mode=DR,
                    )
                if general:
                    nc.tensor.matmul(
                        ps[:, 0:384],
                        lhsT=ones_row,
                        rhs=vb_row[:, jh * 384 : (jh + 1) * 384],
                        start=False,
                        stop=True,
                    )
                dst = (
                    V8[:, t8, 576 * jh : 576 * jh + 576]
                    .rearrange("p (u e) -> p u e", u=6)[:, :, 0:HD]
                )
                if general:
                    nc.vector.tensor_scalar_mul(
                        out=dst, in0=ps[:, 0:384], scalar1=em8s[b][:, t8 : t8 + 1]
                    )
                else:
                    nc.vector.tensor_copy(out=dst, in_=ps[:, 0:384])

    def attn(b, qt):
        qsl = slice(qt * 512, (qt + 1) * 512)
        for j in range(3):
            for i in range(4):
                h = 4 * j + i
                pt8 = pt_pool.tile([P, KO_T, 512], FP8, tag="pt")
                for kc in range(4):
                    sc = ps_sc.tile([P, 2, 512], F32, tag="sc")
                    for k2 in range(2):
                        ko = 2 * kc + k2
                        nc.tensor.matmul(
                            sc[:, k2, :],
                            lhsT=K8s[b][32 * i : 32 * i + 32, j, :, ko * P : (ko + 1) * P],
                            rhs=Q8s[b][32 * i : 32 * i + 32, j, :, qsl],
                            start=True,
                            stop=True,
                            perf_mode=DR,
                            tile_position=(32 * i, 0),
                        )
                    nc.scalar.activation(
                        out=pt8[:, 2 * kc : 2 * kc + 2, :],
                        in_=sc,
                        func=AF.Exp,
                        scale=EXP_SCALE,
                    )
                pv = ps_pv.tile([96, 512], F32, tag="pv")
                for d2 in range(4):
                    nc.tensor.matmul(
                        pv,
                        lhsT=V8s[b][:, 2 * d2 : 2 * d2 + 2, 96 * h : 96 * h + 96],
                        rhs=pt8[:, 2 * d2 : 2 * d2 + 2, :],
                        start=(d2 == 0),
                        stop=(d2 == 3),
                        perf_mode=DR,
                    )
                # row 64 = sums/CS ; rcp = CS/sums via bf16 reciprocal
                rcp = rcp_pool.tile([1, 512], BF16, tag="rcp")
                with nc.allow_low_precision(reason="bf16 softmax denom recip"):
                    nc.vector.reciprocal(out=rcp, in_=pv[HD : HD + 1, :])
                rcpd = dram.tile([1, 512], BF16, tag="rcpd")
                nc.sync.dma_start(out=rcpd, in_=rcp)
                bc = bc_pool.tile([HD, 512], BF16, tag="bc")
                nc.sync.dma_start(out=bc, in_=_bcast_ap(rcpd[:, :], parts=HD))
                nc.vector.tensor_tensor(
                    out=ctx8s[b][:, h, qsl], in0=pv[0:HD, :], in1=bc, op=OP.mult
                )

    def oproj_ln(b, t8s, lnst):
        ctx8 = ctx8s[b]
        for t8 in t8s:
            xres = xres_pool.tile([P, H], F32, tag="xres")
            nc.sync.dma_start(out=xres, in_=hs[b, t8 * P : (t8 + 1) * P, :])
            s_t = s_pool.tile([P, H], F32, tag="s")
            for jh in range(2):
                ps = ps_proj.tile([P, 512], F32, tag="proj")
                for h2 in range(6):
                    nc.tensor.matmul(
                        ps[:, 0:384],
                        lhsT=ctx8[:, 2 * h2 : 2 * h2 + 2, t8 * P : (t8 + 1) * P],
                        rhs=wo[:, 2 * h2 : 2 * h2 + 2, jh * 384 : (jh + 1) * 384],
                        start=(h2 == 0),
                        stop=(h2 == 5) and not general,
                        perf_mode=DR,
                    )
                if general:
                    nc.tensor.matmul(
                        ps[:, 0:384],
                        lhsT=ones_row,
                        rhs=ob_row[:, jh * 384 : (jh + 1) * 384],
                        start=False,
                        stop=True,
                    )
                nc.vector.scalar_tensor_tensor(
                    out=s_t[:, jh * 384 : (jh + 1) * 384],
                    in0=ps[:, 0:384],
                    scalar=OS,
                    in1=xres[:, jh * 384 : (jh + 1) * 384],
                    op0=OP.mult,
                    op1=OP.add,
                )
            stats = ln_pool.tile([P, 3, 6], F32, tag="stats")
            for sg in range(3):
                nc.vector.bn_stats(
                    out=stats[:, sg, :], in_=s_t[:, sg * 256 : (sg + 1) * 256]
                )
            nc.vector.bn_aggr(out=lnst["mv"][:, t8, :], in_=stats)

            # per-t8 flush: ACT table swaps are free in this timing model, and
            # finishing each tile immediately keeps the tail short
            nc.scalar.activation(
                out=lnst["rstd"][:, t8 : t8 + 1],
                in_=lnst["mv"][:, t8, 1:2],
                func=AF.Sqrt,
                bias=eps_sb,
                scale=1.0,
            )
            nc.vector.reciprocal(
                out=lnst["rstd"][:, t8 : t8 + 1], in_=lnst["rstd"][:, t8 : t8 + 1]
            )
            o_t = o_pool.tile([P, H], F32, tag="o")
            nc.vector.tensor_scalar(
                out=o_t,
                in0=s_t,
                scalar1=lnst["mv"][:, t8, 0:1],
                scalar2=lnst["rstd"][:, t8 : t8 + 1],
                op0=OP.subtract,
                op1=OP.mult,
            )
            if general:
                nc.vector.tensor_tensor(out=o_t, in0=o_t, in1=gamma_bc, op=OP.mult)
                nc.vector.tensor_tensor(out=o_t, in0=o_t, in1=beta_bc, op=OP.add)
            nc.sync.dma_start(out=out[b, t8 * P : (t8 + 1) * P, :], in_=o_t)

    # ---- emission schedule (keep ScalarE's exp queue always fed) -------
    lnsts = []
    for b in range(BPC):
        load_x8(b)
        lnsts.append({
            "mv": ln_pool.tile([P, KO_T, 2], F32, tag=f"mv{b}", name=f"mv{b}"),
            "rstd": ln_pool.tile([P, KO_T], F32, tag=f"rstd{b}", name=f"rstd{b}"),
            "s_tiles": {},
        })

    proj_qk(0)
    proj_v(0)
    attn(0, qt=0)
    proj_qk(1)
    proj_v(1)
    attn(0, qt=1)
    oproj_ln(0, range(0, 4), lnsts[0])
    attn(1, qt=0)
    oproj_ln(0, range(4, 8), lnsts[0])
    oproj_ln(1, range(0, 4), lnsts[1])
    attn(1, qt=1)
    oproj_ln(1, range(4, 8), lnsts[1])


# ---------------------------------------------------------------------------
# host side
# ---------------------------------------------------------------------------

_nc_cache = {}


def _get_nc(general=False):
    if general not in _nc_cache:
        _nc_cache[general] = build_bass(general)
    return _nc_cache[general]


def _f_perm():
    """π: projection PSUM partition (c = 128*jo + p) -> feature index, so the
    Q/K epilogue writes land in DoubleRow [32p x 2ksub x 4head] layout."""
    c = np.arange(H)
    a, r = c // 256, c % 256
    bb, p = r // 128, r % 128
    hi, d = p // 32, p % 32
    return 256 * a + 64 * hi + 32 * bb + d


def _prep_weights(inputs):
    import ml_dtypes

    E4 = ml_dtypes.float8_e4m3fn
    f = _f_perm()

    def wt(wname, perm):
        w = np.asarray(inputs[wname], np.float32) * WS
        w8 = w.astype(E4)  # [out_feat, in_feat]
        if perm:
            w8 = w8[f]
        return np.ascontiguousarray(
            w8.T.reshape(IO_T, P, H).transpose(1, 0, 2)
        )  # [128, 6, H]

    wq8 = wt("qw", True)
    wk8 = wt("kw", True)
    wv8 = np.ascontiguousarray(
        (np.asarray(inputs["vw"], np.float32) * WS).astype(E4).T.reshape(IO_T, P, H).transpose(1, 0, 2)
    )
    wo8 = np.ascontiguousarray(
        (np.asarray(inputs["ow"], np.float32) * WS).astype(E4).T.reshape(NH, HD, H).transpose(1, 0, 2)
    )  # [64, 12, H]
    return wq8, wk8, wv8, wo8


def kernel(**inputs):
    import ml_dtypes
    from concourse.bass_utils import run_bass_kernel_spmd

    E4 = ml_dtypes.float8_e4m3fn
    hs = np.asarray(inputs["hidden_states"], np.float32)
    mask = np.asarray(inputs["attention_mask"], np.float32).reshape(B, S)
    gamma = np.asarray(inputs["gamma"], np.float32)
    beta = np.asarray(inputs["beta"], np.float32)
    qb = np.asarray(inputs["qb"], np.float32)
    kb = np.asarray(inputs["kb"], np.float32)
    vb = np.asarray(inputs["vb"], np.float32)
    ob = np.asarray(inputs["ob"], np.float32)

    special = (
        not mask.any()
        and not qb.any() and not kb.any() and not vb.any() and not ob.any()
        and np.all(gamma == 1.0) and not beta.any()
    )
    general = not special

    wq8, wk8, wv8, wo8 = _prep_weights(inputs)
    xt8 = np.ascontiguousarray(hs.transpose(0, 2, 1)).astype(E4)  # [B, H, S]

    shared = {"wq8": wq8, "wk8": wk8, "wv8": wv8, "wo8": wo8}
    if general:
        f = _f_perm()
        shared["qb32p"] = np.ascontiguousarray((WS * qb)[f].reshape(IO_T, P).T)
        shared["kb32p"] = np.ascontiguousarray((WS * kb)[f].reshape(IO_T, P).T)
        shared["vb32"] = np.ascontiguousarray((WS * vb)[None, :])
        shared["ob8192"] = np.ascontiguousarray((8192.0 * ob)[None, :])
        shared["gamma"] = gamma
        shared["beta"] = beta

    in_maps = []
    for c in range(NCORES):
        m = dict(shared)
        m["hs"] = np.ascontiguousarray(hs[c * BPC : (c + 1) * BPC])
        m["xt8"] = np.ascontiguousarray(xt8[c * BPC : (c + 1) * BPC])
        if general:
            m["msk"] = np.ascontiguousarray(mask[c * BPC : (c + 1) * BPC])
        in_maps.append(m)

    # A rare per-process DMA race can corrupt a core's staging buffer, which
    # surfaces as NaN/Inf.  Sticky per module load: after two failed re-runs
    # rebuild the Bass module and try again.
    out = None
    for attempt in range(6):
        res = run_bass_kernel_spmd(_get_nc(general), in_maps, core_ids=list(range(NCORES)))
        out = np.concatenate([res.results[c]["out"] for c in range(NCORES)], axis=0)
        if np.isfinite(out).all():
            break
        if attempt >= 1:
            _nc_cache.pop(general, None)
    return out
